# revision 10
# baseline (speedup 1.0000x reference)
import sys
import numpy as np

if '/opt/trn_rl_repo' not in sys.path:
    sys.path.insert(0, '/opt/trn_rl_repo')

D_MODEL = 512
D_STATE = 32
D_CONV = 4
D_INNER = 1024
DT_RANK = 32
BATCH, SEQ = 8, 1024
N_LAYERS = 4
P = 128
KD = D_MODEL // P      # 4 k-tiles over d_model
JD = D_INNER // P      # 8 tiles over d_inner
MCHUNK = 512           # matmul free-dim chunk (one PSUM bank)

_PROGRAM_CACHE = {}


def build_program(seq=SEQ, n_layers=N_LAYERS, debug=False):
    """Build the per-core Bass program (SPMD; each core owns one batch)."""
    import concourse.bacc as bacc
    import concourse.mybir as mybir
    import concourse.tile as tile
    from contextlib import ExitStack

    alu = mybir.AluOpType
    act = mybir.ActivationFunctionType
    f32 = mybir.dt.float32
    bf16 = mybir.dt.bfloat16

    nchunk = (seq + MCHUNK - 1) // MCHUNK
    chunks = [(c * MCHUNK, min(MCHUNK, seq - c * MCHUNK)) for c in range(nchunk)]

    nc = bacc.Bacc("TRN2", target_bir_lowering=False, debug=debug)

    # ---- DRAM I/O ----
    xT_d = nc.dram_tensor("xT", [KD, P, seq], f32, kind="ExternalInput")
    out_d = nc.dram_tensor("out", [KD, P, seq], f32, kind="ExternalOutput")
    wd = {}
    for l in range(n_layers):
        wd[f"w_in_{l}"] = nc.dram_tensor(f"w_in_{l}", [KD, P, 2 * D_INNER], bf16, kind="ExternalInput")
        wd[f"w_x_{l}"] = nc.dram_tensor(f"w_x_{l}", [JD, P, DT_RANK + 2 * D_STATE], bf16, kind="ExternalInput")
        wd[f"w_dt_{l}"] = nc.dram_tensor(f"w_dt_{l}", [DT_RANK, D_INNER], bf16, kind="ExternalInput")
        wd[f"w_out_{l}"] = nc.dram_tensor(f"w_out_{l}", [JD, P, D_MODEL], bf16, kind="ExternalInput")
        wd[f"cw_{l}"] = nc.dram_tensor(f"cw_{l}", [P, JD * D_CONV], f32, kind="ExternalInput")
        wd[f"cb_{l}"] = nc.dram_tensor(f"cb_{l}", [P, JD], f32, kind="ExternalInput")
        wd[f"dtb_{l}"] = nc.dram_tensor(f"dtb_{l}", [P, JD], f32, kind="ExternalInput")
        wd[f"Dp_{l}"] = nc.dram_tensor(f"Dp_{l}", [P, JD], f32, kind="ExternalInput")
        wd[f"g_{l}"] = nc.dram_tensor(f"g_{l}", [P, KD], f32, kind="ExternalInput")
        wd[f"bb_{l}"] = nc.dram_tensor(f"bb_{l}", [P, KD], f32, kind="ExternalInput")
    # internal scratch
    xs_d = [nc.dram_tensor(f"x_scratch{i}", [KD, P, seq], f32) for i in range(2)]
    bc_d = nc.dram_tensor("bc_scratch", [2 * D_STATE, seq], bf16)
    z_d = nc.dram_tensor("z_scratch", [JD, P, seq], bf16)

    with tile.TileContext(nc) as tc, ExitStack() as ctx:
        pool = ctx.enter_context(tc.tile_pool(name="main", bufs=1))
        pool2 = ctx.enter_context(tc.tile_pool(name="trans", bufs=2))
        pool3 = ctx.enter_context(tc.tile_pool(name="scan", bufs=2))
        wpool = ctx.enter_context(tc.tile_pool(name="wpool", bufs=1))
        statp = ctx.enter_context(tc.tile_pool(name="statp", bufs=1))
        psum = ctx.enter_context(tc.tile_pool(name="psum", bufs=2, space="PSUM"))
        psum_s = ctx.enter_context(tc.tile_pool(name="psum_s", bufs=1, space="PSUM"))

        ones_col = pool.tile([P, 1], f32, tag="ones_col")    # stat-sum lhsT
        nc.vector.memset(ones_col[:], 1.0)
        ones_row = pool.tile([1, P], f32, tag="ones_row")    # replication lhsT
        nc.vector.memset(ones_row[:], 1.0)

        for l in range(n_layers):
            # ---- small weights for this layer ----
            cw = wpool.tile([P, JD * D_CONV], f32, tag="cw")
            nc.sync.dma_start(cw[:], wd[f"cw_{l}"].ap())
            cb = wpool.tile([P, JD], f32, tag="cb")
            nc.sync.dma_start(cb[:], wd[f"cb_{l}"].ap())
            dtb = wpool.tile([P, JD], f32, tag="dtb")
            nc.sync.dma_start(dtb[:], wd[f"dtb_{l}"].ap())
            Dp = wpool.tile([P, JD], f32, tag="Dp")
            nc.sync.dma_start(Dp[:], wd[f"Dp_{l}"].ap())
            gg = wpool.tile([P, KD], f32, tag="gg")
            nc.sync.dma_start(gg[:], wd[f"g_{l}"].ap())
            bb = wpool.tile([P, KD], f32, tag="bb")
            nc.sync.dma_start(bb[:], wd[f"bb_{l}"].ap())

            w_in = wpool.tile([P, KD * 2 * D_INNER], bf16, tag="w_in")
            for kk in range(KD):
                nc.sync.dma_start(w_in[:, kk * 2 * D_INNER:(kk + 1) * 2 * D_INNER], wd[f"w_in_{l}"].ap()[kk])
            w_x = wpool.tile([P, JD * 96], bf16, tag="w_x")
            for kk in range(JD):
                nc.sync.dma_start(w_x[:, kk * 96:(kk + 1) * 96], wd[f"w_x_{l}"].ap()[kk])
            w_dt = wpool.tile([DT_RANK, D_INNER], bf16, tag="w_dt")
            nc.sync.dma_start(w_dt[:], wd[f"w_dt_{l}"].ap())
            w_out = wpool.tile([P, JD * D_MODEL], bf16, tag="w_out")
            for kk in range(JD):
                nc.sync.dma_start(w_out[:, kk * D_MODEL:(kk + 1) * D_MODEL], wd[f"w_out_{l}"].ap()[kk])

            # ---- load x (f32 residual base) and bf16 cast for matmul ----
            x_src = xT_d.ap() if l == 0 else xs_d[(l - 1) % 2].ap()
            xb = pool2.tile([P, KD * seq], bf16, tag="xb")
            for kk in range(KD):
                for (c0, cl) in chunks:
                    xf = pool2.tile([P, MCHUNK], f32, tag="xin")
                    nc.sync.dma_start(xf[:, :cl], x_src[kk, :, c0:c0 + cl])
                    nc.scalar.copy(xb[:, kk * seq + c0:kk * seq + c0 + cl], xf[:, :cl])

            # ---- in_proj -> xm_pre (m 0..7) and z (m 8..15) ----
            xmpre = pool.tile([P, JD * (seq + D_CONV - 1)], bf16, tag="big")
            spre = seq + D_CONV - 1
            for m in range(2 * JD):
                pt = psum.tile([P, MCHUNK], f32, tag="mm")
                for (c0, cl) in chunks:
                    for kk in range(KD):
                        nc.tensor.matmul(
                            pt[:, :cl],
                            w_in[:, kk * 2 * D_INNER + m * P: kk * 2 * D_INNER + (m + 1) * P],
                            xb[:, kk * seq + c0:kk * seq + c0 + cl],
                            start=(kk == 0), stop=(kk == KD - 1))
                    if m < JD:
                        nc.scalar.copy(xmpre[:, m * spre + 3 + c0: m * spre + 3 + c0 + cl], pt[:, :cl])
                    else:
                        mm = m - JD
                        zt = pool2.tile([P, MCHUNK], bf16, tag="zt")
                        nc.scalar.copy(zt[:, :cl], pt[:, :cl])
                        nc.sync.dma_start(z_d.ap()[mm, :, c0:c0 + cl], zt[:, :cl])
            for j in range(JD):
                nc.vector.memset(xmpre[:, j * spre: j * spre + 3], 0.0)

            # ---- depthwise causal conv + silu -> xm (bf16) ----
            # silu(v) = v * sigmoid(v)  (Silu not implemented in CoreSim)
            xm = pool.tile([P, JD * seq], bf16, tag="xm")
            for j in range(JD):
                acc = statp.tile([P, seq], f32, tag="convacc")
                base = j * spre
                nc.vector.tensor_scalar(acc[:], xmpre[:, base:base + seq], cw[:, j * D_CONV:j * D_CONV + 1],
                                        cb[:, j:j + 1], alu.mult, alu.add)
                for k in range(1, D_CONV):
                    nc.vector.scalar_tensor_tensor(
                        acc[:], xmpre[:, base + k:base + k + seq], cw[:, j * D_CONV + k:j * D_CONV + k + 1], acc[:],
                        alu.mult, alu.add)
                sg = statp.tile([P, seq], f32, tag="convsg")
                nc.scalar.activation(sg[:], acc[:], act.Sigmoid)
                nc.vector.tensor_mul(xm[:, j * seq:(j + 1) * seq], acc[:], sg[:])

            # ---- x_proj -> bc96 (dt | B | C), bf16 ----
            bc96 = pool.tile([96, seq], bf16, tag="bc96")
            for (c0, cl) in chunks:
                pt = psum.tile([96, MCHUNK], f32, tag="mm96")
                for kk in range(JD):
                    nc.tensor.matmul(
                        pt[:, :cl], w_x[:, kk * 96:(kk + 1) * 96], xm[:, kk * seq + c0:kk * seq + c0 + cl],
                        start=(kk == 0), stop=(kk == JD - 1))
                nc.scalar.copy(bc96[:, c0:c0 + cl], pt[:, :cl])
            nc.sync.dma_start(bc_d.ap(), bc96[DT_RANK:96, :])

            # ---- dt_proj + softplus -> delta (bf16); du = delta*xm ----
            delta = pool.tile([P, JD * seq], bf16, tag="delta")
            for j in range(JD):
                for (c0, cl) in chunks:
                    pt = psum.tile([P, MCHUNK], f32, tag="mm")
                    nc.tensor.matmul(pt[:, :cl], w_dt[:, j * P:(j + 1) * P], bc96[0:DT_RANK, c0:c0 + cl],
                                     start=True, stop=True)
                    # softplus(v) = ln(exp(v) + 1)  (no Softplus in ACT tables)
                    ex = pool2.tile([P, MCHUNK], f32, tag="sp_exp")
                    nc.scalar.activation(ex[:, :cl], pt[:, :cl], act.Exp, bias=dtb[:, j:j + 1])
                    nc.scalar.activation(delta[:, j * seq + c0: j * seq + c0 + cl], ex[:, :cl],
                                         act.Ln, bias=ones_col[:])

            # ---- selective scan ----
            y_sb = pool.tile([P, JD * seq], f32, tag="big")
            for n in range(D_STATE):
                Brep = pool2.tile([P, seq], bf16, tag="Brep")
                nc.sync.dma_start(Brep[:], bc_d.ap()[n:n + 1, :].partition_broadcast(P)[:, 0, :])
                Crep = pool2.tile([P, seq], bf16, tag="Crep")
                nc.sync.dma_start(Crep[:], bc_d.ap()[D_STATE + n:D_STATE + n + 1, :].partition_broadcast(P)[:, 0, :])
                for j in range(JD):
                    dA = pool3.tile([P, seq], bf16, tag="dA")
                    nc.scalar.activation(dA[:], delta[:, j * seq:(j + 1) * seq], act.Exp, scale=-float(n + 1))
                    dBu = pool3.tile([P, seq], bf16, tag="dBu")
                    nc.gpsimd.tensor_mul(dBu[:], delta[:, j * seq:(j + 1) * seq], Brep[:])
                    nc.gpsimd.tensor_mul(dBu[:], dBu[:], xm[:, j * seq:(j + 1) * seq])
                    h = pool3.tile([P, seq], bf16, tag="h")
                    nc.vector.tensor_tensor_scan(h[:], dA[:], dBu[:], 0.0, alu.mult, alu.add)
                    ys = y_sb[:, j * seq:(j + 1) * seq]
                    if n == 0:
                        nc.vector.tensor_mul(ys, h[:], Crep[:])
                    else:
                        nc.vector.tensor_mul(dBu[:], h[:], Crep[:])
                        nc.vector.tensor_add(ys, ys, dBu[:])

            # ---- y = (y + xm*D) * silu(z); cast bf16 ----
            yg = pool.tile([P, JD * seq], bf16, tag="yg")
            for j in range(JD):
                ys = y_sb[:, j * seq:(j + 1) * seq]
                nc.vector.scalar_tensor_tensor(ys, xm[:, j * seq:(j + 1) * seq], Dp[:, j:j + 1], ys,
                                               alu.mult, alu.add)
                zj = pool2.tile([P, seq], bf16, tag="zj")
                nc.sync.dma_start(zj[:], z_d.ap()[j])
                sz = statp.tile([P, seq], f32, tag="sz")
                nc.scalar.activation(sz[:], zj[:], act.Sigmoid)
                nc.vector.tensor_mul(sz[:], sz[:], zj[:])
                nc.vector.tensor_mul(yg[:, j * seq:(j + 1) * seq], ys, sz[:])

            # ---- out_proj + residual -> o (f32) ----
            o_sb = pool.tile([P, KD * seq], f32, tag="big")
            for m in range(KD):
                for (c0, cl) in chunks:
                    pt = psum.tile([P, MCHUNK], f32, tag="mm")
                    for kk in range(JD):
                        nc.tensor.matmul(
                            pt[:, :cl], w_out[:, kk * D_MODEL + m * P: kk * D_MODEL + (m + 1) * P],
                            yg[:, kk * seq + c0:kk * seq + c0 + cl],
                            start=(kk == 0), stop=(kk == JD - 1))
                    res = pool2.tile([P, MCHUNK], f32, tag="res")
                    nc.sync.dma_start(res[:, :cl], x_src[m, :, c0:c0 + cl])
                    nc.vector.tensor_add(o_sb[:, m * seq + c0:m * seq + c0 + cl], pt[:, :cl], res[:, :cl])

            # ---- layernorm over d_model (partition dim, 4 tiles), per chunk ----
            last = (l == n_layers - 1)
            for (c0, cl) in chunks:
                psum_sum = psum_s.tile([1, MCHUNK], f32, tag="stat_sum")
                psum_sq = psum_s.tile([1, MCHUNK], f32, tag="stat_sq")
                for m in range(KD):
                    sq1 = pool2.tile([P, MCHUNK], f32, tag="sq1")
                    nc.scalar.activation(sq1[:, :cl], o_sb[:, m * seq + c0:m * seq + c0 + cl], act.Square)
                    nc.tensor.matmul(psum_sum[:, :cl], ones_col[:], o_sb[:, m * seq + c0:m * seq + c0 + cl],
                                     start=(m == 0), stop=(m == KD - 1))
                    nc.tensor.matmul(psum_sq[:, :cl], ones_col[:], sq1[:, :cl],
                                     start=(m == 0), stop=(m == KD - 1))
                mean = statp.tile([1, MCHUNK], f32, tag="mean")
                nc.scalar.activation(mean[:, :cl], psum_sum[:, :cl], act.Copy, scale=1.0 / D_MODEL)
                msq = statp.tile([1, MCHUNK], f32, tag="msq")
                nc.scalar.activation(msq[:, :cl], psum_sq[:, :cl], act.Copy, scale=1.0 / D_MODEL)
                var = statp.tile([1, MCHUNK], f32, tag="var")
                nc.vector.tensor_mul(var[:, :cl], mean[:, :cl], mean[:, :cl])
                nc.vector.tensor_sub(var[:, :cl], msq[:, :cl], var[:, :cl])
                nc.vector.tensor_scalar(var[:, :cl], var[:, :cl], 1e-5, None, alu.add)
                std = statp.tile([1, MCHUNK], f32, tag="std")
                nc.scalar.activation(std[:, :cl], var[:, :cl], act.Sqrt)
                rstd = statp.tile([1, MCHUNK], f32, tag="rstd")
                nc.vector.reciprocal(rstd[:, :cl], std[:, :cl])

                mrep = psum_s.tile([P, MCHUNK], f32, tag="mrep")
                rrep = psum_s.tile([P, MCHUNK], f32, tag="rrep")
                nc.tensor.matmul(mrep[:, :cl], ones_row[:], mean[:, :cl], start=True, stop=True)
                nc.tensor.matmul(rrep[:, :cl], ones_row[:], rstd[:, :cl], start=True, stop=True)

                for m in range(KD):
                    t1 = pool2.tile([P, MCHUNK], f32, tag="ln1")
                    nc.vector.tensor_sub(t1[:, :cl], o_sb[:, m * seq + c0:m * seq + c0 + cl], mrep[:, :cl])
                    nc.vector.tensor_mul(t1[:, :cl], t1[:, :cl], rrep[:, :cl])
                    xo = pool2.tile([P, MCHUNK], f32, tag="xout")
                    nc.vector.tensor_scalar(xo[:, :cl], t1[:, :cl], gg[:, m:m + 1], bb[:, m:m + 1], alu.mult, alu.add)
                    dst = out_d.ap() if last else xs_d[l % 2].ap()
                    nc.sync.dma_start(dst[m, :, c0:c0 + cl], xo[:, :cl])

    nc.compile()
    return nc


def prep_weights(params, n_layers=N_LAYERS):
    """Host-side weight transforms -> dict of np arrays (shared across cores)."""
    import ml_dtypes
    bf = ml_dtypes.bfloat16
    w = {}
    for l in range(n_layers):
        p = params[l]
        get = lambda k: np.asarray(p[k], dtype=np.float32)
        w[f"w_in_{l}"] = np.ascontiguousarray(
            get('in_proj_w').T.reshape(KD, P, 2 * D_INNER)).astype(bf)
        w[f"w_x_{l}"] = np.ascontiguousarray(
            get('x_proj_w').T.reshape(JD, P, 96)).astype(bf)
        w[f"w_dt_{l}"] = np.ascontiguousarray(get('dt_proj_w').T).astype(bf)
        w[f"w_out_{l}"] = np.ascontiguousarray(
            get('out_proj_w').T.reshape(JD, P, D_MODEL)).astype(bf)
        # conv_w (Din,1,4) -> [P, JD*4] with col j*4+k = w[j*128+p, k]
        cwf = get('conv_w').reshape(D_INNER, D_CONV).reshape(JD, P, D_CONV)
        w[f"cw_{l}"] = np.ascontiguousarray(cwf.transpose(1, 0, 2).reshape(P, JD * D_CONV))
        w[f"cb_{l}"] = np.ascontiguousarray(get('conv_b').reshape(JD, P).T)
        w[f"dtb_{l}"] = np.ascontiguousarray(get('dt_proj_b').reshape(JD, P).T)
        w[f"Dp_{l}"] = np.ascontiguousarray(get('D').reshape(JD, P).T)
        w[f"g_{l}"] = np.ascontiguousarray(get('ln_g').reshape(KD, P).T)
        w[f"bb_{l}"] = np.ascontiguousarray(get('ln_b').reshape(KD, P).T)
    return w


def kernel(x, params):
    from concourse.bass_utils import run_bass_kernel_spmd

    x = np.asarray(x, dtype=np.float32)
    B, L, DM = x.shape
    key = (L, len(params))
    if key not in _PROGRAM_CACHE:
        _PROGRAM_CACHE[key] = build_program(seq=L, n_layers=len(params))
    nc = _PROGRAM_CACHE[key]

    w = prep_weights(params, n_layers=len(params))
    in_maps = []
    for b in range(B):
        xb = np.ascontiguousarray(x[b].T.reshape(KD, P, L))
        m = {"xT": xb}
        m.update(w)
        in_maps.append(m)
    res = run_bass_kernel_spmd(nc, in_maps, list(range(B)))
    out = np.empty((B, L, DM), np.float32)
    for b in range(B):
        o = res.results[b]["out"]          # [KD, P, L]
        out[b] = o.reshape(DM, L).T
    return out
